# revision 15
# baseline (speedup 1.0000x reference)
"""GAT (2-layer, PyG-style) distributed Bass kernel for 8 TRN2 NeuronCores.

Strategy (1D node partition by dst, v2 "batched-gather" design):
  - core c owns dst nodes [c*NPC, (c+1)*NPC).
  - dense phase: each core computes table rows [h(64)|asrc(8)|adst(8)] for its
    node slice into a 256B-strided local table; AllGather -> full table.
  - edge phase: edges sorted by (src-table-quarter, dst).  positions are
    (quarter, dst) pairs; blocks = 128 consecutive positions (= 128 nodes of
    one quarter); tiles = up to 128 edges of one block.  Per supertile
    (K tiles): ONE dma_gather per quarter-run fetches all src rows (144B
    payload), ONE dma_gather fetches per-edge a_dst (16B payload, dst is
    core-local so indices fit int16).  w = exp(leakyrelu(asrc+adst)) on-chip,
    rhs = [h*w | w]; one matmul per tile scatters edges into the block's
    PSUM [128 pos, nw] via a one-hot built on DVE.  Finished blocks are
    copied to SBUF and accumulate-DMA'd (SWDGE accum_op=add) into a per-node
    f32 accumulator (the 4 quarter contributions of a node sum in DRAM).
  - normalize phases read the accumulator node-ordered, divide by the
    denominator columns, apply bias/ELU (+W2 matmul for layer 1, log_softmax
    for layer 2) and write the next table / output contiguously (no scatter).
All floating-point math runs on-device; host work is integer graph
preprocessing and weight layout rearrangement.
"""
import os
import sys
import numpy as np

try:
    import concourse.bass as bass
except ImportError:  # pragma: no cover
    for p in ("/opt/trn_rl_repo", "/root/.axon_site/_ro/trn_rl_repo"):
        if os.path.isdir(p) and p not in sys.path:
            sys.path.insert(0, p)
    import concourse.bass as bass

import ml_dtypes
import concourse.mybir as mybir
import concourse.tile as tile
import concourse.bacc as bacc
from concourse.masks import make_identity

BF16 = ml_dtypes.bfloat16
DT = mybir.dt

# ---------------- problem config (hardcoded per contract) ----------------
N, E, F = 100000, 1600000, 256
H1, C1 = 8, 8          # layer1 heads x channels (concat -> 64)
C2 = 40                # layer2 single head, 40 classes
NEG = 0.2
NCORES = 8
NPC = N // NCORES      # 12500 owned nodes per core
NPCP = 12544           # padded rows per core (98*128)
BN = 112               # nodes per block (112*112 = 12544)
NBPQ = NPCP // BN      # blocks per quarter (112)
QS = 2 * NPCP          # table rows per quarter (25088 < 32768, int16-safe)
NQ = 4                 # quarters
RSTR = 128             # table row stride in bf16 elems (256B)
K = 96                 # tiles per supertile
ROW1 = 72              # gathered src row, layer 1: [h(64)|asrc(8)]
AD1 = 8                # gathered dst payload, layer 1 (adst, cols 72:80)
ROW2 = 41              # gathered src row, layer 2: [h2(40)|asrc2(1)]
AD2 = 2                # gathered dst payload, layer 2 (cols 40:42, use col 1)
NW1 = 72               # rhs width layer 1: [h*w(64)|w(8)]
NW2 = 41               # rhs width layer 2: [h2*w(40)|w(1)]
ACC1W = 128            # accum row stride (f32): 512B
ACC2W = 64             # accum row stride layer 2 (f32): 256B

_f32 = np.float32


# =================== host-side graph preprocessing ===================

def preprocess(edge_index):
    """Integer-only graph preprocessing.

    The device program structure (tile -> block map, quarter runs, PSUM
    start/stop, flush groups) is baked into the instruction stream and must
    be IDENTICAL on all 8 cores.  We therefore use a common tiles-per-block
    count (max over cores) and pad per-core data tiles where a core has
    fewer edges in a block.
    """
    src = edge_index[0].astype(np.int64)       # self-loops handled in the
    dst = edge_index[1].astype(np.int64)       # normalize phases instead
    srow = (src // NPC) * NPCP + (src % NPC)   # table row of src node

    NBLK = NQ * NBPQ                            # 448 blocks per core
    cores = []
    for c in range(NCORES):
        lo = c * NPC
        m = (dst >= lo) & (dst < lo + NPC)
        r, d = srow[m], dst[m] - lo
        q = r // QS
        order = np.lexsort((d, q))
        r, d, q = r[order], d[order], q[order]
        blk = (q * NBPQ + d // BN).astype(np.int64)
        bstart = np.searchsorted(blk, np.arange(NBLK), side="left")
        bend = np.searchsorted(blk, np.arange(NBLK), side="right")
        cores.append(dict(r=r, d=d, bstart=bstart, bend=bend))

    ecount = np.stack([c["bend"] - c["bstart"] for c in cores])  # [8, NBLK]
    tpb = np.maximum(((ecount + 127) // 128).max(axis=0), 1)     # common
    tblk = np.repeat(np.arange(NBLK), tpb)
    ntiles = -(-len(tblk) // K) * K
    tblk = np.concatenate([tblk, np.full(ntiles - len(tblk), NBLK - 1)])
    NST = ntiles // K
    tile_off = np.zeros(NBLK + 1, np.int64)
    tile_off[1:] = np.cumsum(tpb)

    # per-supertile quarter runs (common)
    tq = tblk // NBPQ
    runs = []
    for sti in range(NST):
        rr = []
        c0 = 0
        seg = tq[sti * K:(sti + 1) * K]
        for cc in range(1, K + 1):
            if cc == K or seg[cc] != seg[c0]:
                rr.append((int(c0), int(cc), int(seg[c0])))
                c0 = cc
        runs.append(rr)
    meta = dict(NST=NST, runs=runs, tblk=[int(b) for b in tblk])

    per_core = []
    for c in cores:
        r, d = c["r"], c["d"]
        sidx = np.zeros((NST, 16, K * 8), np.int16)
        didx = np.zeros((NST, 16, K * 8), np.int16)
        dloc = np.full((NST, 128, K), 200, np.int16)
        for b in range(NBLK):
            a0, e0 = int(c["bstart"][b]), int(c["bend"][b])
            qb, bl = divmod(b, NBPQ)
            for j in range(int(tpb[b])):
                a = a0 + 128 * j
                bnd = min(a + 128, e0)
                if bnd <= a:
                    break
                ti = int(tile_off[b]) + j
                sti, cc = divmod(ti, K)
                n = bnd - a
                ii = cc * 128 + np.arange(n)
                sidx[sti, ii % 16, ii // 16] = r[a:bnd] - qb * QS
                didx[sti, ii % 16, ii // 16] = d[a:bnd]
                dloc[sti, 0:n, cc] = d[a:bnd] - BN * bl
        per_core.append(dict(
            sidx=np.ascontiguousarray(np.tile(sidx, (1, 8, 1))),
            didx=np.ascontiguousarray(np.tile(didx, (1, 8, 1))),
            dloc=dloc.astype(BF16)))
    return meta, per_core


def build_weight_inputs(W1, att_src1, att_dst1, bias1, W2, att_src2, att_dst2,
                        bias2):
    """Pure layout rearrangement of weights (no FP arithmetic)."""
    A1 = np.zeros((64, 16), _f32)
    for h in range(H1):
        A1[h * 8:(h + 1) * 8, h] = att_src1[h]
        A1[h * 8:(h + 1) * 8, 8 + h] = att_dst1[h]
    att2 = np.concatenate([att_src2.T, att_dst2.T], axis=1).astype(_f32)
    b1r = np.broadcast_to(bias1.astype(_f32), (128, 64)).copy()
    b2r = np.broadcast_to(bias2.astype(_f32), (128, C2)).copy()
    return dict(W1=W1.astype(_f32), A1=A1, W2=W2.astype(_f32), att2=att2,
                b1r=b1r, b2r=b2r)


# =================== device program ===================

def _brd(ap, pattern, off=0):
    """Manual AP: keep partition dim, explicit free-dim [step,count] pattern."""
    return bass.AP(ap.tensor, ap.offset + off, [ap.ap[0]] + pattern)


def dma_gather_raw(nc, out_ap, in_ap, idxs_ap, num_idxs, elem_size, elem_step,
                   queue_num=0):
    """bass.dma_gather without the %256 elem_size restriction."""
    g = nc.gpsimd
    stride_bytes = elem_step * mybir.dt.size(in_ap.dtype)
    assert stride_bytes % 256 == 0
    _in_ap = g.lower_ap_dma(in_ap, for_custom_bir_dma=True)
    _idxs_ap = g.lower_ap(idxs_ap)
    _out_ap = g.lower_ap(out_ap)
    return g.add_instruction(
        mybir.InstDMAGatherAnt(
            name=nc.get_next_instruction_name(),
            ins=[*_in_ap, _idxs_ap, g.lower_val_access(g.to_reg(num_idxs))],
            outs=[_out_ap],
            transpose=False, num_idxs=num_idxs, elem_size=elem_size,
            stride_bytes_256=stride_bytes // 256, gen_mode=0,
            single_packet=False, queue_num=queue_num,
            sbuf_tokens_per_rank=0, sbuf_free_dim_per_rank=0,
            sbuf_free_dim_pad_per_rank=0, sbuf_byte_offset=0))


def build_program(meta):
    NT = NPCP * NCORES
    n_ptile = NPCP // 128
    NSTMAX = meta["NST"]

    nc = bacc.Bacc("TRN2", target_bir_lowering=False, debug=False,
                   enable_asserts=False, num_devices=NCORES,
                   num_swdge_queues=4)

    def din(name, shape, dt):
        return nc.dram_tensor(name, shape, dt, kind="ExternalInput").ap()

    x_sl = din("x_sl", [NPCP, F], DT.float32)
    W1 = din("W1", [F, 64], DT.float32)
    A1 = din("A1", [64, 16], DT.float32)
    W2 = din("W2", [64, C2], DT.float32)
    att2 = din("att2", [C2, 2], DT.float32)
    b1r = din("b1r", [128, 64], DT.float32)
    b2r = din("b2r", [128, C2], DT.float32)
    sidx_d = din("sidx", [NSTMAX, 128, K * 8], DT.int16)
    didx_d = din("didx", [NSTMAX, 128, K * 8], DT.int16)
    dloc_d = din("dloc", [NSTMAX, 128, K], DT.bfloat16)

    out_d = nc.dram_tensor("out", [NPCP, C2], DT.float32,
                           kind="ExternalOutput").ap()

    t1loc = nc.dram_tensor("t1loc", [NPCP, RSTR], DT.bfloat16).ap()
    t1full = nc.dram_tensor("t1full", [NT, RSTR], DT.bfloat16,
                            addr_space="Shared").ap()
    t2loc = nc.dram_tensor("t2loc", [NPCP, RSTR], DT.bfloat16).ap()
    t2full = nc.dram_tensor("t2full", [NT, RSTR], DT.bfloat16,
                            addr_space="Shared").ap()
    acc1 = nc.dram_tensor("acc1", [NPCP, ACC1W], DT.float32).ap()
    acc2 = nc.dram_tensor("acc2", [NPCP, ACC2W], DT.float32).ap()

    groups = [list(range(NCORES))]

    with tile.TileContext(nc, num_cores=NCORES) as tc:
        from contextlib import ExitStack
        with ExitStack() as top:
            cpool = top.enter_context(tc.tile_pool(name="const", bufs=1))
            id_f = cpool.tile([128, 128], DT.float32)
            make_identity(nc, id_f[:])
            id_b = cpool.tile([128, 128], DT.bfloat16)
            nc.vector.tensor_copy(id_b[:], id_f[:])
            iota128 = cpool.tile([128, 128], DT.bfloat16)
            iota_i = cpool.tile([128, 128], DT.int16)
            nc.gpsimd.iota(iota_i[:], pattern=[[1, 128]], base=0,
                           channel_multiplier=0)
            nc.vector.tensor_copy(iota128[:], iota_i[:])
            b1sb = cpool.tile([128, 64], DT.float32)
            nc.sync.dma_start(b1sb[:], b1r)
            b2sb = cpool.tile([128, C2], DT.float32)
            nc.sync.dma_start(b2sb[:], b2r)

            # ---------- P0: weight prep ----------
            rhs1 = [cpool.tile([128, 80], DT.bfloat16, tag=f"rhs1_{i}",
                               name=f"rhs1_{i}") for i in range(2)]
            rhs2 = cpool.tile([64, 42], DT.bfloat16)
            with tc.tile_pool(name="p0", bufs=1) as p0, \
                 tc.tile_pool(name="p0ps", bufs=1, space="PSUM") as p0ps:
                w1sb = [p0.tile([128, 64], DT.float32, tag=f"w1_{i}",
                                name=f"w1_{i}") for i in range(2)]
                for i in range(2):
                    nc.sync.dma_start(w1sb[i][:], W1[128 * i:128 * (i + 1), :])
                a1sb = p0.tile([64, 16], DT.float32)
                nc.sync.dma_start(a1sb[:], A1)
                w2sb = p0.tile([64, C2], DT.float32)
                nc.sync.dma_start(w2sb[:], W2)
                at2sb = p0.tile([C2, 2], DT.float32)
                nc.sync.dma_start(at2sb[:], att2)
                for i in range(2):
                    tp = p0ps.tile([64, 128], DT.float32, tag="w1t_ps")
                    nc.tensor.transpose(tp[:], w1sb[i][:], id_f[:])
                    w1t = p0.tile([64, 128], DT.float32, tag="w1t")
                    nc.vector.tensor_copy(w1t[:], tp[:])
                    wa = p0ps.tile([128, 16], DT.float32, tag="w1a_ps")
                    nc.tensor.matmul(wa[:], lhsT=w1t[:], rhs=a1sb[:],
                                     start=True, stop=True)
                    nc.vector.tensor_copy(rhs1[i][:, 0:64], w1sb[i][:])
                    nc.vector.tensor_copy(rhs1[i][:, 64:80], wa[:])
                tp2 = p0ps.tile([C2, 64], DT.float32, tag="w2t_ps")
                nc.tensor.transpose(tp2[:], w2sb[:], id_f[:64, :64])
                w2t = p0.tile([C2, 64], DT.float32)
                nc.vector.tensor_copy(w2t[:], tp2[:])
                wa2 = p0ps.tile([64, 2], DT.float32, tag="w2a_ps")
                nc.tensor.matmul(wa2[:], lhsT=w2t[:], rhs=at2sb[:],
                                 start=True, stop=True)
                nc.vector.tensor_copy(rhs2[:, 0:C2], w2sb[:])
                nc.vector.tensor_copy(rhs2[:, C2:C2 + 2], wa2[:])

            # ---------- P0.5: zero accumulators ----------
            with tc.tile_pool(name="pz", bufs=1) as pz:
                zt = pz.tile([128, ACC1W * n_ptile], DT.float32)
                nc.vector.memset(zt[:], 0.0)
                nc.sync.dma_start(
                    bass.AP(acc1.tensor, 0,
                            [[ACC1W * n_ptile, 128], [1, ACC1W * n_ptile]]),
                    zt[:])
                nc.sync.dma_start(
                    bass.AP(acc2.tensor, 0,
                            [[ACC2W * n_ptile, 128], [1, ACC2W * n_ptile]]),
                    zt[:, 0:ACC2W * n_ptile])

            # ---------- P1: dense layer-1 table ----------
            with tc.tile_pool(name="p1", bufs=3) as p1, \
                 tc.tile_pool(name="p1ps", bufs=2, space="PSUM") as p1ps:
                for it in range(n_ptile):
                    xt = p1.tile([128, F], DT.float32, tag="x")
                    nc.sync.dma_start(xt[:], x_sl[128 * it:128 * (it + 1), :])
                    xb = p1.tile([128, F], DT.bfloat16, tag="xb")
                    nc.vector.tensor_copy(xb[:], xt[:])
                    xT = p1.tile([128, F], DT.bfloat16, tag="xT")
                    ps1 = p1ps.tile([128, 80], DT.float32, tag="ps1")
                    for i in range(2):
                        tp = p1ps.tile([128, 128], DT.bfloat16, tag="xt_ps")
                        nc.tensor.transpose(
                            tp[:], xb[:, 128 * i:128 * (i + 1)], id_b[:])
                        nc.scalar.copy(xT[:, 128 * i:128 * (i + 1)], tp[:])
                    for i in range(2):
                        nc.tensor.matmul(
                            ps1[:], lhsT=xT[:, 128 * i:128 * (i + 1)],
                            rhs=rhs1[i][:], start=(i == 0), stop=(i == 1))
                    st = p1.tile([128, 80], DT.bfloat16, tag="st1")
                    nc.scalar.copy(st[:], ps1[:])
                    nc.sync.dma_start(
                        bass.AP(t1loc.tensor, it * 128 * RSTR,
                                [[RSTR, 128], [1, 80]]), st[:])

            # ---------- P2: AllGather table1 ----------
            nc.gpsimd.collective_compute(
                "AllGather", mybir.AluOpType.bypass, replica_groups=groups,
                ins=[t1loc.opt()], outs=[t1full.opt()])

            # ---------- P3: edge pass layer 1 ----------
            rep = int(os.environ.get("GAT_EDGE_REPEAT", "1"))
            for _ in range(rep):
                edge_pass(nc, tc, meta, 1, sidx_d, didx_d, dloc_d,
                          t1full, t1loc, acc1, iota128)

            # ---------- P4: normalize L1 -> table2 ----------
            with tc.tile_pool(name="p4", bufs=3) as p4, \
                 tc.tile_pool(name="p4ps", bufs=2, space="PSUM") as p4ps:
                for it in range(n_ptile):
                    ac = p4.tile([128, ROW1], DT.float32, tag="ac")
                    nc.sync.dma_start(
                        ac[:], bass.AP(acc1.tensor, it * 128 * ACC1W,
                                       [[ACC1W, 128], [1, ROW1]]))
                    # self-loop: acc += [w_self * h | w_self]
                    tr = p4.tile([128, 80], DT.bfloat16, tag="tr")
                    nc.sync.dma_start(
                        tr[:], bass.AP(t1loc.tensor, it * 128 * RSTR,
                                       [[RSTR, 128], [1, 80]]))
                    es = p4.tile([128, 8], DT.float32, tag="es")
                    nc.vector.tensor_tensor(out=es[:], in0=tr[:, 64:72],
                                            in1=tr[:, 72:80],
                                            op=mybir.AluOpType.add)
                    et = p4.tile([128, 8], DT.float32, tag="et")
                    nc.vector.tensor_scalar_mul(et[:], es[:], NEG)
                    nc.vector.tensor_tensor(out=es[:], in0=es[:], in1=et[:],
                                            op=mybir.AluOpType.max)
                    ws = p4.tile([128, 8], DT.float32, tag="ws")
                    nc.scalar.activation(ws[:], es[:],
                                         mybir.ActivationFunctionType.Exp)
                    wh = p4.tile([128, 64], DT.float32, tag="wh")
                    nc.vector.tensor_tensor(
                        out=wh[:], in0=tr[:, 0:64],
                        in1=_brd(ws[:], [[1, 8], [0, 8]]),
                        op=mybir.AluOpType.mult)
                    nc.vector.tensor_tensor(out=ac[:, 0:64], in0=ac[:, 0:64],
                                            in1=wh[:],
                                            op=mybir.AluOpType.add)
                    nc.vector.tensor_tensor(out=ac[:, 64:72],
                                            in0=ac[:, 64:72], in1=ws[:],
                                            op=mybir.AluOpType.add)
                    den = p4.tile([128, 8], DT.float32, tag="den")
                    nc.vector.tensor_scalar_max(den[:], ac[:, 64:72], 1e-30)
                    rec = p4.tile([128, 8], DT.float32, tag="rec")
                    nc.vector.reciprocal(rec[:], den[:])
                    hin = p4.tile([128, 64], DT.float32, tag="hin")
                    for h in range(H1):
                        nc.vector.tensor_scalar(
                            out=hin[:, 8 * h:8 * (h + 1)],
                            in0=ac[:, 8 * h:8 * (h + 1)],
                            scalar1=rec[:, h:h + 1], scalar2=None,
                            op0=mybir.AluOpType.mult)
                    nc.vector.tensor_tensor(out=hin[:], in0=hin[:],
                                            in1=b1sb[:],
                                            op=mybir.AluOpType.add)
                    emn = p4.tile([128, 64], DT.float32, tag="emn")
                    nc.vector.tensor_scalar_min(emn[:], hin[:], 0.0)
                    nc.scalar.activation(emn[:], emn[:],
                                         mybir.ActivationFunctionType.Exp)
                    nc.vector.tensor_scalar_max(hin[:], hin[:], 0.0)
                    nc.vector.tensor_tensor(out=hin[:], in0=hin[:],
                                            in1=emn[:],
                                            op=mybir.AluOpType.add)
                    helu = p4.tile([128, 64], DT.bfloat16, tag="helu")
                    nc.vector.tensor_scalar_add(helu[:], hin[:], -1.0)
                    htp = p4ps.tile([64, 128], DT.bfloat16, tag="htp")
                    nc.tensor.transpose(htp[:], helu[:], id_b[:])
                    hts = p4.tile([64, 128], DT.bfloat16, tag="hts")
                    nc.scalar.copy(hts[:], htp[:])
                    h2ps = p4ps.tile([128, 42], DT.float32, tag="h2ps")
                    nc.tensor.matmul(h2ps[:], lhsT=hts[:], rhs=rhs2[:],
                                     start=True, stop=True)
                    st2 = p4.tile([128, 42], DT.bfloat16, tag="st2")
                    nc.scalar.copy(st2[:], h2ps[:])
                    nc.sync.dma_start(
                        bass.AP(t2loc.tensor, it * 128 * RSTR,
                                [[RSTR, 128], [1, 42]]), st2[:])

            # ---------- P5: AllGather table2 ----------
            nc.gpsimd.collective_compute(
                "AllGather", mybir.AluOpType.bypass, replica_groups=groups,
                ins=[t2loc.opt()], outs=[t2full.opt()])

            # ---------- P6: edge pass layer 2 ----------
            for _ in range(rep):
                edge_pass(nc, tc, meta, 2, sidx_d, didx_d, dloc_d,
                          t2full, t2loc, acc2, iota128)

            # ---------- P7: normalize L2 -> log_softmax out ----------
            with tc.tile_pool(name="p7", bufs=3) as p7:
                for it in range(n_ptile):
                    ac = p7.tile([128, NW2], DT.float32, tag="ac2")
                    nc.sync.dma_start(
                        ac[:], bass.AP(acc2.tensor, it * 128 * ACC2W,
                                       [[ACC2W, 128], [1, NW2]]))
                    tr = p7.tile([128, 42], DT.bfloat16, tag="tr2")
                    nc.sync.dma_start(
                        tr[:], bass.AP(t2loc.tensor, it * 128 * RSTR,
                                       [[RSTR, 128], [1, 42]]))
                    es = p7.tile([128, 1], DT.float32, tag="es2")
                    nc.vector.tensor_tensor(out=es[:], in0=tr[:, 40:41],
                                            in1=tr[:, 41:42],
                                            op=mybir.AluOpType.add)
                    et = p7.tile([128, 1], DT.float32, tag="et2")
                    nc.vector.tensor_scalar_mul(et[:], es[:], NEG)
                    nc.vector.tensor_tensor(out=es[:], in0=es[:], in1=et[:],
                                            op=mybir.AluOpType.max)
                    ws = p7.tile([128, 1], DT.float32, tag="ws2")
                    nc.scalar.activation(ws[:], es[:],
                                         mybir.ActivationFunctionType.Exp)
                    wh = p7.tile([128, C2], DT.float32, tag="wh2")
                    nc.vector.tensor_scalar(
                        out=wh[:], in0=tr[:, 0:C2], scalar1=ws[:],
                        scalar2=None, op0=mybir.AluOpType.mult)
                    nc.vector.tensor_tensor(out=ac[:, 0:C2], in0=ac[:, 0:C2],
                                            in1=wh[:],
                                            op=mybir.AluOpType.add)
                    nc.vector.tensor_tensor(out=ac[:, C2:C2 + 1],
                                            in0=ac[:, C2:C2 + 1], in1=ws[:],
                                            op=mybir.AluOpType.add)
                    den = p7.tile([128, 1], DT.float32, tag="d2")
                    nc.vector.tensor_scalar_max(den[:], ac[:, C2:C2 + 1],
                                                1e-30)
                    rec = p7.tile([128, 1], DT.float32, tag="r2")
                    nc.vector.reciprocal(rec[:], den[:])
                    o2 = p7.tile([128, C2], DT.float32, tag="o2")
                    nc.vector.tensor_scalar(
                        out=o2[:], in0=ac[:, 0:C2], scalar1=rec[:],
                        scalar2=None, op0=mybir.AluOpType.mult)
                    nc.vector.tensor_tensor(out=o2[:], in0=o2[:], in1=b2sb[:],
                                            op=mybir.AluOpType.add)
                    mx = p7.tile([128, 1], DT.float32, tag="mx")
                    nc.vector.tensor_reduce(mx[:], o2[:],
                                            axis=mybir.AxisListType.X,
                                            op=mybir.AluOpType.max)
                    z = p7.tile([128, C2], DT.float32, tag="z")
                    nc.vector.tensor_scalar(
                        out=z[:], in0=o2[:], scalar1=mx[:], scalar2=None,
                        op0=mybir.AluOpType.subtract)
                    ez = p7.tile([128, C2], DT.float32, tag="ez")
                    se = p7.tile([128, 1], DT.float32, tag="se")
                    nc.scalar.activation(ez[:], z[:],
                                         mybir.ActivationFunctionType.Exp,
                                         accum_out=se[:])
                    lse = p7.tile([128, 1], DT.float32, tag="lse")
                    nc.scalar.activation(lse[:], se[:],
                                         mybir.ActivationFunctionType.Ln)
                    zo = p7.tile([128, C2], DT.float32, tag="zo")
                    nc.vector.tensor_scalar(
                        out=zo[:], in0=z[:], scalar1=lse[:], scalar2=None,
                        op0=mybir.AluOpType.subtract)
                    nc.sync.dma_start(out_d[128 * it:128 * (it + 1), :],
                                      zo[:])

    nc.compile()
    return nc


def edge_pass(nc, tc, meta, layer, sidx_d, didx_d, dloc_d, tfull, tloc,
              acc, iota128):
    ablate = set(os.environ.get("GAT_ABLATE", "").split(","))
    NST = meta["NST"]
    runs = meta["runs"]
    tblk = meta["tblk"]
    row = ROW1 if layer == 1 else ROW2          # src gather width
    adw = AD1 if layer == 1 else AD2            # dst gather width
    adcol = 72 if layer == 1 else 40            # dst gather start col
    nh = H1 if layer == 1 else 1
    nw = NW1 if layer == 1 else NW2
    accw = ACC1W if layer == 1 else ACC2W

    ntile_tot = NST * K
    # start/stop flags per tile
    start_f = [i == 0 or tblk[i] != tblk[i - 1] for i in range(ntile_tot)]
    stop_f = [i == ntile_tot - 1 or tblk[i] != tblk[i + 1]
              for i in range(ntile_tot)]
    # block -> flush group (groups of <=8 consecutive blocks, same quarter)
    def grp(b):
        q, bl = divmod(b, NBPQ)
        return q * ((NBPQ + 7) // 8) + bl // 8
    def grp_span(g):
        ng = (NBPQ + 7) // 8
        q, gl = divmod(g, ng)
        lo = gl * 8
        return q, lo, min(lo + 8, NBPQ)
    last_blk_of_grp = {}
    for i in range(ntile_tot):
        last_blk_of_grp[grp(tblk[i])] = tblk[i]

    from contextlib import ExitStack
    with ExitStack() as ctx:
        pm = ctx.enter_context(tc.tile_pool(name=f"e{layer}m", bufs=2))
        pg = ctx.enter_context(tc.tile_pool(name=f"e{layer}g", bufs=3))
        pw = ctx.enter_context(tc.tile_pool(name=f"e{layer}w", bufs=2))
        pb = ctx.enter_context(tc.tile_pool(name=f"e{layer}b", bufs=2))
        ps_blk = ctx.enter_context(
            tc.tile_pool(name=f"e{layer}ps", bufs=4, space="PSUM"))
        state = {}
        fix = {}
        if ablate & {"nogather", "novec"}:
            pfix = ctx.enter_context(
                tc.tile_pool(name=f"e{layer}fx", bufs=1))
            if "nogather" in ablate:
                fix["hs"] = pfix.tile([128, K * row], DT.bfloat16, name=f"fxhs{layer}")
                nc.vector.memset(fix["hs"][:], 0.5)
                fix["ad"] = pfix.tile([128, K * adw], DT.bfloat16, name=f"fxad{layer}")
                nc.vector.memset(fix["ad"][:], 0.5)
            if "novec" in ablate:
                fix["hw"] = pfix.tile([128, K * nw], DT.bfloat16, name=f"fxhw{layer}")
                nc.vector.memset(fix["hw"][:], 0.25)
                fix["spos"] = pfix.tile([128, K * 128], DT.bfloat16, name=f"fxsp{layer}")
                nc.vector.memset(fix["spos"][:], 0.0)

        def mm_stage(s, spos, hw):
            # per-tile matmul into the block PSUM; flush groups via accum-DMA
            for c in range(K):
                t = s * K + c
                b = tblk[t]
                if start_f[t]:
                    blkps = ps_blk.tile([128, nw], DT.float32, tag="blkps")
                    state["ps"] = blkps
                nc.tensor.matmul(
                    state["ps"][:],
                    lhsT=spos[:, 128 * c:128 * (c + 1)],
                    rhs=hw[:, nw * c:nw * (c + 1)],
                    start=start_f[t], stop=stop_f[t], skip_group_check=True)
                if stop_f[t]:
                    g = grp(b)
                    q, lo, hi = grp_span(g)
                    gt = state.get("g")
                    if gt is None:
                        gt = pb.tile([128, 8 * nw], DT.float32, tag="gacc",
                                     name=f"gacc_{layer}_{g}")
                        state["g"] = gt
                    bl = b % NBPQ
                    if "noflush" not in ablate:
                        nc.scalar.copy(
                            gt[:, nw * (bl - lo):nw * (bl - lo + 1)],
                            state["ps"][:])
                    if b == last_blk_of_grp[g]:
                        gs = hi - lo
                        gap = bass.AP(
                            gt[:].tensor, gt[:].offset,
                            [[gt[:].ap[0][0], BN], [nw, gs], [1, nw]])
                        if "noflush" not in ablate:
                            nc.gpsimd.dma_start(
                                bass.AP(acc.tensor, lo * BN * accw,
                                        [[accw, BN], [BN * accw, gs], [1, nw]]),
                                gap, accum_op=mybir.AluOpType.add)
                        state["g"] = None

        for s in range(NST):
            sidx = pm.tile([128, K * 8], DT.int16, tag="sidx")
            nc.sync.dma_start(sidx[:], sidx_d[s])
            didx = pm.tile([128, K * 8], DT.int16, tag="didx")
            nc.sync.dma_start(didx[:], didx_d[s])
            dl = pm.tile([128, K], DT.bfloat16, tag="dl")
            nc.sync.dma_start(dl[:], dloc_d[s])

            # src gather (per quarter run) + dst gather (one call)
            if "nogather" in ablate:
                hs, ad = fix["hs"], fix["ad"]
            else:
                hs = pg.tile([128, K * row], DT.bfloat16, tag="hs")
                for (c0, c1, q) in runs[s]:
                    # split each run across the 4 SWDGE queues: desc-gen/ring
                    # throughput is per-queue (~10.5 ns/desc serial)
                    n128 = c1 - c0
                    bnd = [c0 + (n128 * i) // 4 for i in range(5)]
                    for qi in range(4):
                        a, b = bnd[qi], bnd[qi + 1]
                        if b > a:
                            dma_gather_raw(
                                nc,
                                _brd(hs[:], [[row, b - a], [1, row]],
                                     off=row * a),
                                bass.AP(tfull.tensor, q * QS * RSTR,
                                        [[RSTR, QS], [1, row]]),
                                sidx[:, 8 * a:8 * b], (b - a) * 128, row,
                                RSTR, queue_num=qi)
                ad = pg.tile([128, K * adw], DT.bfloat16, tag="ad")
                for qi in range(4):
                    a, b = (K * qi) // 4, (K * (qi + 1)) // 4
                    dma_gather_raw(
                        nc, _brd(ad[:], [[adw, b - a], [1, adw]],
                                 off=adw * a),
                        bass.AP(tloc.tensor, adcol, [[RSTR, NPCP], [1, adw]]),
                        didx[:, 8 * a:8 * b], (b - a) * 128, adw, RSTR,
                        queue_num=qi)

            # e = asrc + adst ; leakyrelu ; w = exp(e)
            if "novec" in ablate:
                if "nomm" not in ablate:
                    mm_stage(s, fix["spos"], fix["hw"])
                continue
            e = pw.tile([128, K * nh], DT.float32, tag="e")
            if layer == 1:
                nc.vector.tensor_tensor(
                    out=_brd(e[:], [[nh, K], [1, nh]]),
                    in0=_brd(hs[:], [[row, K], [1, nh]], off=64),
                    in1=_brd(ad[:], [[adw, K], [1, nh]]),
                    op=mybir.AluOpType.add)
            else:
                nc.vector.tensor_tensor(
                    out=_brd(e[:], [[nh, K], [1, nh]]),
                    in0=_brd(hs[:], [[row, K], [1, 1]], off=40),
                    in1=_brd(ad[:], [[adw, K], [1, 1]], off=1),
                    op=mybir.AluOpType.add)
            tmp = pw.tile([128, K * nh], DT.float32, tag="etmp")
            nc.vector.tensor_scalar_mul(tmp[:], e[:], NEG)
            nc.vector.tensor_tensor(out=e[:], in0=e[:], in1=tmp[:],
                                    op=mybir.AluOpType.max)
            w = pw.tile([128, K * nh], DT.bfloat16, tag="w")
            nc.scalar.activation(w[:], e[:], mybir.ActivationFunctionType.Exp)

            # rhs = [h*w | w]
            hw = pw.tile([128, K * nw], DT.bfloat16, tag="hw")
            if layer == 1:
                nc.vector.tensor_tensor(
                    out=_brd(hw[:], [[nw, K], [8, 8], [1, 8]]),
                    in0=_brd(hs[:], [[row, K], [8, 8], [1, 8]]),
                    in1=_brd(w[:], [[nh, K], [1, 8], [0, 8]]),
                    op=mybir.AluOpType.mult)
                nc.vector.tensor_copy(
                    _brd(hw[:], [[nw, K], [1, 8]], off=64), w[:])
            else:
                nc.vector.tensor_tensor(
                    out=_brd(hw[:], [[nw, K], [1, C2]]),
                    in0=_brd(hs[:], [[row, K], [1, C2]]),
                    in1=_brd(w[:], [[1, K], [0, C2]]),
                    op=mybir.AluOpType.mult)
                nc.vector.tensor_copy(
                    _brd(hw[:], [[nw, K], [1, 1]], off=C2), w[:])

            # one-hot spos[slot, j*128+q] = (dl[slot,j] == q)
            spos = pw.tile([128, K * 128], DT.bfloat16, tag="spos")
            nc.vector.tensor_tensor(
                out=_brd(spos[:], [[128, K], [1, 128]]),
                in0=_brd(iota128[:], [[0, K], [1, 128]]),
                in1=_brd(dl[:], [[1, K], [0, 128]]),
                op=mybir.AluOpType.is_equal)

            if "nomm" not in ablate:
                mm_stage(s, spos, hw)


# =================== SPMD runner (bass2jax-based, with timing) ===================

def _run_spmd(nc, in_maps, n_timing_iters=0):
    """Execute the program on NCORES neuron devices via PJRT (axon)."""
    import jax
    from jax.sharding import Mesh, PartitionSpec
    from jax.experimental.shard_map import shard_map
    from concourse import bass2jax
    from concourse.bass2jax import _bass_exec_p, partition_id_tensor
    import time

    bass2jax.install_neuronx_cc_hook()
    assert nc.dbg_addr is None or not nc.dbg_callbacks

    in_names, out_names, out_avals, zero_outs = [], [], [], []
    partition_name = (nc.partition_id_tensor.name
                      if nc.partition_id_tensor else None)
    for alloc in nc.m.functions[0].allocations:
        if not isinstance(alloc, mybir.MemoryLocationSet):
            continue
        name = alloc.memorylocations[0].name
        if alloc.kind == "ExternalInput":
            if name != partition_name:
                in_names.append(name)
        elif alloc.kind == "ExternalOutput":
            out_names.append(name)
            shape = tuple(alloc.tensor_shape)
            dtype = mybir.dt.np(alloc.dtype)
            out_avals.append(jax.core.ShapedArray(shape, dtype))
            zero_outs.append(np.zeros(shape, dtype))
    n_params = len(in_names)
    all_in_names = in_names + out_names + (
        [partition_name] if partition_name else [])

    def _body(*args):
        operands = list(args)
        if partition_name is not None:
            operands.append(partition_id_tensor())
        return tuple(_bass_exec_p.bind(
            *operands,
            out_avals=tuple(out_avals),
            in_names=tuple(all_in_names),
            out_names=tuple(out_names),
            lowering_input_output_aliases=(),
            sim_require_finite=True,
            sim_require_nnan=True,
            nc=nc,
        ))

    devices = jax.devices()[:NCORES]
    mesh = Mesh(np.asarray(devices), ("core",))
    nin = n_params + len(out_names)
    fn = jax.jit(shard_map(_body, mesh=mesh,
                           in_specs=(PartitionSpec("core"),) * nin,
                           out_specs=(PartitionSpec("core"),) * len(out_names),
                           check_rep=False),
                 keep_unused=True)
    sh = jax.sharding.NamedSharding(mesh, PartitionSpec("core"))
    concat_in = [
        jax.device_put(np.concatenate(
            [np.asarray(in_maps[c][name]) for c in range(NCORES)], axis=0), sh)
        for name in in_names
    ]
    concat_zeros = [
        jax.device_put(np.zeros((NCORES * z.shape[0], *z.shape[1:]), z.dtype),
                       sh) for z in zero_outs
    ]
    out_arrs = jax.block_until_ready(fn(*concat_in, *concat_zeros))
    times = []
    for _ in range(n_timing_iters):
        t0 = time.perf_counter()
        r = jax.block_until_ready(fn(*concat_in, *concat_zeros))
        times.append(time.perf_counter() - t0)
        del r
    results = [
        {name: np.asarray(out_arrs[i]).reshape(NCORES, *out_avals[i].shape)[c]
         for i, name in enumerate(out_names)}
        for c in range(NCORES)
    ]
    return results, times


# =================== top-level entry ===================

def kernel(**inputs):
    edge_index = np.asarray(inputs["edge_index"])
    meta, per_core = preprocess(edge_index)
    wts = build_weight_inputs(
        np.asarray(inputs["W1"]), np.asarray(inputs["att_src1"]),
        np.asarray(inputs["att_dst1"]), np.asarray(inputs["bias1"]),
        np.asarray(inputs["W2"]), np.asarray(inputs["att_src2"]),
        np.asarray(inputs["att_dst2"]), np.asarray(inputs["bias2"]))
    x = np.asarray(inputs["x"], _f32)
    in_maps = []
    for c in range(NCORES):
        pc = per_core[c]
        xs = np.zeros((NPCP, F), _f32)
        xs[:NPC] = x[c * NPC:(c + 1) * NPC]
        in_maps.append(dict(
            x_sl=xs, W1=wts["W1"], A1=wts["A1"], W2=wts["W2"],
            att2=wts["att2"], b1r=wts["b1r"], b2r=wts["b2r"],
            sidx=pc["sidx"], didx=pc["didx"], dloc=pc["dloc"]))
    nc = build_program(meta)
    n_iters = int(os.environ.get("GAT_BENCH_ITERS", "0"))
    results, times = _run_spmd(nc, in_maps, n_timing_iters=n_iters)
    global LAST_TIMES
    LAST_TIMES = times
    out = np.zeros((N, C2), _f32)
    for c in range(NCORES):
        out[c * NPC:(c + 1) * NPC] = results[c]["out"][:NPC]
    return out


LAST_TIMES = []



# revision 27
# speedup vs baseline: 1.8705x; 1.8705x over previous
"""GAT (2-layer, PyG-style) distributed Bass kernel for 8 TRN2 NeuronCores.

Strategy (1D node partition by dst, v2 "batched-gather" design):
  - core c owns dst nodes [c*NPC, (c+1)*NPC).
  - dense phase: each core computes table rows [h(64)|asrc(8)|adst(8)] for its
    node slice into a 256B-strided local table; AllGather -> full table.
  - edge phase: edges sorted by (src-table-quarter, dst).  positions are
    (quarter, dst) pairs; blocks = 128 consecutive positions (= 128 nodes of
    one quarter); tiles = up to 128 edges of one block.  Per supertile
    (K tiles): ONE dma_gather per quarter-run fetches all src rows (144B
    payload), ONE dma_gather fetches per-edge a_dst (16B payload, dst is
    core-local so indices fit int16).  w = exp(leakyrelu(asrc+adst)) on-chip,
    rhs = [h*w | w]; one matmul per tile scatters edges into the block's
    PSUM [128 pos, nw] via a one-hot built on DVE.  Finished blocks are
    copied to SBUF and accumulate-DMA'd (SWDGE accum_op=add) into a per-node
    f32 accumulator (the 4 quarter contributions of a node sum in DRAM).
  - normalize phases read the accumulator node-ordered, divide by the
    denominator columns, apply bias/ELU (+W2 matmul for layer 1, log_softmax
    for layer 2) and write the next table / output contiguously (no scatter).
All floating-point math runs on-device; host work is integer graph
preprocessing and weight layout rearrangement.
"""
import os
import sys
import numpy as np

try:
    import concourse.bass as bass
except ImportError:  # pragma: no cover
    for p in ("/opt/trn_rl_repo", "/root/.axon_site/_ro/trn_rl_repo"):
        if os.path.isdir(p) and p not in sys.path:
            sys.path.insert(0, p)
    import concourse.bass as bass

import ml_dtypes
import concourse.mybir as mybir
import concourse.tile as tile
import concourse.bacc as bacc
from concourse.masks import make_identity

BF16 = ml_dtypes.bfloat16
DT = mybir.dt

# ---------------- problem config (hardcoded per contract) ----------------
N, E, F = 100000, 1600000, 256
H1, C1 = 8, 8          # layer1 heads x channels (concat -> 64)
C2 = 40                # layer2 single head, 40 classes
NEG = 0.2
NCORES = 8
NPC = N // NCORES      # 12500 owned nodes per core
NPCP = 12544           # padded rows per core (98*128)
BN = 112               # nodes per block (112*112 = 12544)
NBPQ = NPCP // BN      # blocks per quarter (112)
QS = 2 * NPCP          # table rows per quarter (25088 < 32768, int16-safe)
NQ = 4                 # quarters
RSTR = 128             # table row stride in bf16 elems (256B)
K = 96                 # tiles per supertile
ROW1 = 72              # gathered src row, layer 1: [h(64)|asrc(8)]
AD1 = 8                # gathered dst payload, layer 1 (adst, cols 72:80)
ROW2 = 41              # gathered src row, layer 2: [h2(40)|asrc2(1)]
AD2 = 2                # gathered dst payload, layer 2 (cols 40:42, use col 1)
NW1 = 72               # rhs width layer 1: [h*w(64)|w(8)]
NW2 = 41               # rhs width layer 2: [h2*w(40)|w(1)]
ACC1W = 128            # accum row stride (f32): 512B
ACC2W = 64             # accum row stride layer 2 (f32): 256B

_f32 = np.float32


# =================== host-side graph preprocessing ===================

def preprocess(edge_index):
    """Integer-only graph preprocessing.

    The device program structure (tile -> block map, quarter runs, PSUM
    start/stop, flush groups) is baked into the instruction stream and must
    be IDENTICAL on all 8 cores.  We therefore use a common tiles-per-block
    count (max over cores) and pad per-core data tiles where a core has
    fewer edges in a block.
    """
    src = edge_index[0].astype(np.int64)       # self-loops handled in the
    dst = edge_index[1].astype(np.int64)       # normalize phases instead
    srow = (src // NPC) * NPCP + (src % NPC)   # table row of src node

    NBLK = NQ * NBPQ                            # 448 blocks per core
    cores = []
    for c in range(NCORES):
        lo = c * NPC
        m = (dst >= lo) & (dst < lo + NPC)
        r, d = srow[m], dst[m] - lo
        q = r // QS
        order = np.lexsort((d, q))
        r, d, q = r[order], d[order], q[order]
        blk = (q * NBPQ + d // BN).astype(np.int64)
        bstart = np.searchsorted(blk, np.arange(NBLK), side="left")
        bend = np.searchsorted(blk, np.arange(NBLK), side="right")
        cores.append(dict(r=r, d=d, bstart=bstart, bend=bend))

    ecount = np.stack([c["bend"] - c["bstart"] for c in cores])  # [8, NBLK]
    tpb = np.maximum(((ecount + 127) // 128).max(axis=0), 1)     # common
    tblk = np.repeat(np.arange(NBLK), tpb)
    ntiles = -(-len(tblk) // K) * K
    tblk = np.concatenate([tblk, np.full(ntiles - len(tblk), NBLK - 1)])
    NST = ntiles // K
    tile_off = np.zeros(NBLK + 1, np.int64)
    tile_off[1:] = np.cumsum(tpb)

    # per-supertile quarter runs (common)
    tq = tblk // NBPQ
    runs = []
    for sti in range(NST):
        rr = []
        c0 = 0
        seg = tq[sti * K:(sti + 1) * K]
        for cc in range(1, K + 1):
            if cc == K or seg[cc] != seg[c0]:
                rr.append((int(c0), int(cc), int(seg[c0])))
                c0 = cc
        runs.append(rr)
    meta = dict(NST=NST, runs=runs, tblk=[int(b) for b in tblk])

    per_core = []
    for c in cores:
        r, d = c["r"], c["d"]
        sidx = np.zeros((NST, 16, K * 8), np.int16)
        dloc = np.full((NST, 128, K), 200, np.int16)
        for b in range(NBLK):
            a0, e0 = int(c["bstart"][b]), int(c["bend"][b])
            qb, bl = divmod(b, NBPQ)
            for j in range(int(tpb[b])):
                a = a0 + 128 * j
                bnd = min(a + 128, e0)
                if bnd <= a:
                    break
                ti = int(tile_off[b]) + j
                sti, cc = divmod(ti, K)
                n = bnd - a
                ii = cc * 128 + np.arange(n)
                sidx[sti, ii % 16, ii // 16] = r[a:bnd] - qb * QS
                dloc[sti, 0:n, cc] = d[a:bnd] - BN * bl
        # dlT[s, c*128+slot] = dloc[s, slot, c] (for the transposed one-hot)
        dlT = np.ascontiguousarray(
            dloc.transpose(0, 2, 1).reshape(NST, K * 128))
        per_core.append(dict(
            sidx=np.ascontiguousarray(np.tile(sidx, (1, 8, 1))),
            dloc=dloc.astype(BF16),
            dlT=dlT.astype(BF16)))
    return meta, per_core


def build_weight_inputs(W1, att_src1, att_dst1, bias1, W2, att_src2, att_dst2,
                        bias2):
    """Pure layout rearrangement of weights (no FP arithmetic)."""
    A1 = np.zeros((64, 16), _f32)
    for h in range(H1):
        A1[h * 8:(h + 1) * 8, h] = att_src1[h]
        A1[h * 8:(h + 1) * 8, 8 + h] = att_dst1[h]
    att2 = np.concatenate([att_src2.T, att_dst2.T], axis=1).astype(_f32)
    b1r = np.broadcast_to(bias1.astype(_f32), (128, 64)).copy()
    b2r = np.broadcast_to(bias2.astype(_f32), (128, C2)).copy()
    return dict(W1=W1.astype(_f32), A1=A1, W2=W2.astype(_f32), att2=att2,
                b1r=b1r, b2r=b2r)


# =================== device program ===================

def _brd(ap, pattern, off=0):
    """Manual AP: keep partition dim, explicit free-dim [step,count] pattern."""
    return bass.AP(ap.tensor, ap.offset + off, [ap.ap[0]] + pattern)


def dma_gather_raw(nc, out_ap, in_ap, idxs_ap, num_idxs, elem_size, elem_step,
                   queue_num=0):
    """bass.dma_gather without the %256 elem_size restriction."""
    g = nc.gpsimd
    stride_bytes = elem_step * mybir.dt.size(in_ap.dtype)
    assert stride_bytes % 256 == 0
    _in_ap = g.lower_ap_dma(in_ap, for_custom_bir_dma=True)
    _idxs_ap = g.lower_ap(idxs_ap)
    _out_ap = g.lower_ap(out_ap)
    return g.add_instruction(
        mybir.InstDMAGatherAnt(
            name=nc.get_next_instruction_name(),
            ins=[*_in_ap, _idxs_ap, g.lower_val_access(g.to_reg(num_idxs))],
            outs=[_out_ap],
            transpose=False, num_idxs=num_idxs, elem_size=elem_size,
            stride_bytes_256=stride_bytes // 256, gen_mode=0,
            single_packet=False, queue_num=queue_num,
            sbuf_tokens_per_rank=0, sbuf_free_dim_per_rank=0,
            sbuf_free_dim_pad_per_rank=0, sbuf_byte_offset=0))


def build_program(meta):
    NT = NPCP * NCORES
    n_ptile = NPCP // 128
    NSTMAX = meta["NST"]

    nc = bacc.Bacc("TRN2", target_bir_lowering=False, debug=False,
                   enable_asserts=False, num_devices=NCORES,
                   num_swdge_queues=4)

    def din(name, shape, dt):
        return nc.dram_tensor(name, shape, dt, kind="ExternalInput").ap()

    x_sl = din("x_sl", [NPCP, F], DT.float32)
    W1 = din("W1", [F, 64], DT.float32)
    A1 = din("A1", [64, 16], DT.float32)
    W2 = din("W2", [64, C2], DT.float32)
    att2 = din("att2", [C2, 2], DT.float32)
    b1r = din("b1r", [128, 64], DT.float32)
    b2r = din("b2r", [128, C2], DT.float32)
    sidx_d = din("sidx", [NSTMAX, 128, K * 8], DT.int16)
    dloc_d = din("dloc", [NSTMAX, 128, K], DT.bfloat16)
    dlT_d = din("dlT", [NSTMAX, K * 128], DT.bfloat16)

    out_d = nc.dram_tensor("out", [NPCP, C2], DT.float32,
                           kind="ExternalOutput").ap()

    t1loc = nc.dram_tensor("t1loc", [NPCP, RSTR], DT.bfloat16).ap()
    t1full = nc.dram_tensor("t1full", [NT, RSTR], DT.bfloat16,
                            addr_space="Shared").ap()
    t2loc = nc.dram_tensor("t2loc", [NPCP, RSTR], DT.bfloat16).ap()
    t2full = nc.dram_tensor("t2full", [NT, RSTR], DT.bfloat16,
                            addr_space="Shared").ap()
    acc1 = nc.dram_tensor("acc1", [NPCP, ACC1W], DT.float32).ap()
    acc2 = nc.dram_tensor("acc2", [NPCP, ACC2W], DT.float32).ap()

    groups = [list(range(NCORES))]

    with tile.TileContext(nc, num_cores=NCORES) as tc:
        from contextlib import ExitStack
        with ExitStack() as top:
            cpool = top.enter_context(tc.tile_pool(name="const", bufs=1))
            id_f = cpool.tile([128, 128], DT.float32)
            make_identity(nc, id_f[:])
            id_b = cpool.tile([128, 128], DT.bfloat16)
            nc.vector.tensor_copy(id_b[:], id_f[:])
            iota128 = cpool.tile([128, 128], DT.bfloat16)
            iota_i = cpool.tile([128, 128], DT.int16)
            nc.gpsimd.iota(iota_i[:], pattern=[[1, 128]], base=0,
                           channel_multiplier=0)
            nc.vector.tensor_copy(iota128[:], iota_i[:])
            iota_p = cpool.tile([128, 1], DT.bfloat16)
            iota_pi = cpool.tile([128, 1], DT.int16)
            nc.gpsimd.iota(iota_pi[:], pattern=[[1, 1]], base=0,
                           channel_multiplier=1)
            nc.vector.tensor_copy(iota_p[:], iota_pi[:])
            b1sb = cpool.tile([128, 64], DT.float32)
            nc.sync.dma_start(b1sb[:], b1r)
            b2sb = cpool.tile([128, C2], DT.float32)
            nc.sync.dma_start(b2sb[:], b2r)

            # ---------- P0: weight prep ----------
            rhs1 = [cpool.tile([128, 80], DT.bfloat16, tag=f"rhs1_{i}",
                               name=f"rhs1_{i}") for i in range(2)]
            rhs2 = cpool.tile([64, 42], DT.bfloat16)
            with tc.tile_pool(name="p0", bufs=1) as p0, \
                 tc.tile_pool(name="p0ps", bufs=1, space="PSUM") as p0ps:
                w1sb = [p0.tile([128, 64], DT.float32, tag=f"w1_{i}",
                                name=f"w1_{i}") for i in range(2)]
                for i in range(2):
                    nc.sync.dma_start(w1sb[i][:], W1[128 * i:128 * (i + 1), :])
                a1sb = p0.tile([64, 16], DT.float32)
                nc.sync.dma_start(a1sb[:], A1)
                w2sb = p0.tile([64, C2], DT.float32)
                nc.sync.dma_start(w2sb[:], W2)
                at2sb = p0.tile([C2, 2], DT.float32)
                nc.sync.dma_start(at2sb[:], att2)
                for i in range(2):
                    tp = p0ps.tile([64, 128], DT.float32, tag="w1t_ps")
                    nc.tensor.transpose(tp[:], w1sb[i][:], id_f[:])
                    w1t = p0.tile([64, 128], DT.float32, tag="w1t")
                    nc.vector.tensor_copy(w1t[:], tp[:])
                    wa = p0ps.tile([128, 16], DT.float32, tag="w1a_ps")
                    nc.tensor.matmul(wa[:], lhsT=w1t[:], rhs=a1sb[:],
                                     start=True, stop=True)
                    nc.vector.tensor_copy(rhs1[i][:, 0:64], w1sb[i][:])
                    nc.vector.tensor_copy(rhs1[i][:, 64:80], wa[:])
                tp2 = p0ps.tile([C2, 64], DT.float32, tag="w2t_ps")
                nc.tensor.transpose(tp2[:], w2sb[:], id_f[:64, :64])
                w2t = p0.tile([C2, 64], DT.float32)
                nc.vector.tensor_copy(w2t[:], tp2[:])
                wa2 = p0ps.tile([64, 2], DT.float32, tag="w2a_ps")
                nc.tensor.matmul(wa2[:], lhsT=w2t[:], rhs=at2sb[:],
                                 start=True, stop=True)
                nc.vector.tensor_copy(rhs2[:, 0:C2], w2sb[:])
                nc.vector.tensor_copy(rhs2[:, C2:C2 + 2], wa2[:])

            # ---------- P0.5: zero accumulators ----------
            with tc.tile_pool(name="pz", bufs=1) as pz:
                zt = pz.tile([128, ACC1W * n_ptile], DT.float32)
                nc.vector.memset(zt[:], 0.0)
                nc.sync.dma_start(
                    bass.AP(acc1.tensor, 0,
                            [[ACC1W * n_ptile, 128], [1, ACC1W * n_ptile]]),
                    zt[:])
                nc.sync.dma_start(
                    bass.AP(acc2.tensor, 0,
                            [[ACC2W * n_ptile, 128], [1, ACC2W * n_ptile]]),
                    zt[:, 0:ACC2W * n_ptile])

            # ---------- P1: dense layer-1 table ----------
            with tc.tile_pool(name="p1", bufs=3) as p1, \
                 tc.tile_pool(name="p1ps", bufs=2, space="PSUM") as p1ps:
                for it in range(n_ptile):
                    xt = p1.tile([128, F], DT.float32, tag="x")
                    nc.sync.dma_start(xt[:], x_sl[128 * it:128 * (it + 1), :])
                    xb = p1.tile([128, F], DT.bfloat16, tag="xb")
                    nc.vector.tensor_copy(xb[:], xt[:])
                    xT = p1.tile([128, F], DT.bfloat16, tag="xT")
                    ps1 = p1ps.tile([128, 80], DT.float32, tag="ps1")
                    for i in range(2):
                        tp = p1ps.tile([128, 128], DT.bfloat16, tag="xt_ps")
                        nc.tensor.transpose(
                            tp[:], xb[:, 128 * i:128 * (i + 1)], id_b[:])
                        nc.scalar.copy(xT[:, 128 * i:128 * (i + 1)], tp[:])
                    for i in range(2):
                        nc.tensor.matmul(
                            ps1[:], lhsT=xT[:, 128 * i:128 * (i + 1)],
                            rhs=rhs1[i][:], start=(i == 0), stop=(i == 1))
                    st = p1.tile([128, 80], DT.bfloat16, tag="st1")
                    nc.scalar.copy(st[:], ps1[:])
                    nc.sync.dma_start(
                        bass.AP(t1loc.tensor, it * 128 * RSTR,
                                [[RSTR, 128], [1, 80]]), st[:])

            # ---------- P2: AllGather table1 ----------
            nc.gpsimd.collective_compute(
                "AllGather", mybir.AluOpType.bypass, replica_groups=groups,
                ins=[t1loc.opt()], outs=[t1full.opt()])

            # ---------- P3: edge pass layer 1 ----------
            rep = int(os.environ.get("GAT_EDGE_REPEAT", "1"))
            for _ in range(rep):
                edge_pass(nc, tc, meta, 1, sidx_d, dloc_d, dlT_d,
                          t1full, t1loc, acc1, iota128, iota_p)

            # ---------- P4: normalize L1 -> table2 ----------
            with tc.tile_pool(name="p4", bufs=3) as p4, \
                 tc.tile_pool(name="p4ps", bufs=2, space="PSUM") as p4ps:
                for it in range(n_ptile):
                    ac = p4.tile([128, ROW1], DT.float32, tag="ac")
                    nc.sync.dma_start(
                        ac[:], bass.AP(acc1.tensor, it * 128 * ACC1W,
                                       [[ACC1W, 128], [1, ROW1]]))
                    # self-loop: acc += [w_self * h | w_self]
                    tr = p4.tile([128, 80], DT.bfloat16, tag="tr")
                    nc.sync.dma_start(
                        tr[:], bass.AP(t1loc.tensor, it * 128 * RSTR,
                                       [[RSTR, 128], [1, 80]]))
                    es = p4.tile([128, 8], DT.float32, tag="es")
                    nc.vector.tensor_tensor(out=es[:], in0=tr[:, 64:72],
                                            in1=tr[:, 72:80],
                                            op=mybir.AluOpType.add)
                    et = p4.tile([128, 8], DT.float32, tag="et")
                    nc.vector.tensor_scalar_mul(et[:], es[:], NEG)
                    nc.vector.tensor_tensor(out=es[:], in0=es[:], in1=et[:],
                                            op=mybir.AluOpType.max)
                    ws = p4.tile([128, 8], DT.float32, tag="ws")
                    nc.scalar.activation(ws[:], es[:],
                                         mybir.ActivationFunctionType.Exp)
                    wh = p4.tile([128, 64], DT.float32, tag="wh")
                    nc.vector.tensor_tensor(
                        out=wh[:], in0=tr[:, 0:64],
                        in1=_brd(ws[:], [[1, 8], [0, 8]]),
                        op=mybir.AluOpType.mult)
                    nc.vector.tensor_tensor(out=ac[:, 0:64], in0=ac[:, 0:64],
                                            in1=wh[:],
                                            op=mybir.AluOpType.add)
                    nc.vector.tensor_tensor(out=ac[:, 64:72],
                                            in0=ac[:, 64:72], in1=ws[:],
                                            op=mybir.AluOpType.add)
                    den = p4.tile([128, 8], DT.float32, tag="den")
                    nc.vector.tensor_scalar_max(den[:], ac[:, 64:72], 1e-30)
                    rec = p4.tile([128, 8], DT.float32, tag="rec")
                    nc.vector.reciprocal(rec[:], den[:])
                    hin = p4.tile([128, 64], DT.float32, tag="hin")
                    for h in range(H1):
                        nc.vector.tensor_scalar(
                            out=hin[:, 8 * h:8 * (h + 1)],
                            in0=ac[:, 8 * h:8 * (h + 1)],
                            scalar1=rec[:, h:h + 1], scalar2=None,
                            op0=mybir.AluOpType.mult)
                    nc.vector.tensor_tensor(out=hin[:], in0=hin[:],
                                            in1=b1sb[:],
                                            op=mybir.AluOpType.add)
                    emn = p4.tile([128, 64], DT.float32, tag="emn")
                    nc.vector.tensor_scalar_min(emn[:], hin[:], 0.0)
                    nc.scalar.activation(emn[:], emn[:],
                                         mybir.ActivationFunctionType.Exp)
                    nc.vector.tensor_scalar_max(hin[:], hin[:], 0.0)
                    nc.vector.tensor_tensor(out=hin[:], in0=hin[:],
                                            in1=emn[:],
                                            op=mybir.AluOpType.add)
                    helu = p4.tile([128, 64], DT.bfloat16, tag="helu")
                    nc.vector.tensor_scalar_add(helu[:], hin[:], -1.0)
                    htp = p4ps.tile([64, 128], DT.bfloat16, tag="htp")
                    nc.tensor.transpose(htp[:], helu[:], id_b[:])
                    hts = p4.tile([64, 128], DT.bfloat16, tag="hts")
                    nc.scalar.copy(hts[:], htp[:])
                    h2ps = p4ps.tile([128, 42], DT.float32, tag="h2ps")
                    nc.tensor.matmul(h2ps[:], lhsT=hts[:], rhs=rhs2[:],
                                     start=True, stop=True)
                    st2 = p4.tile([128, 42], DT.bfloat16, tag="st2")
                    nc.scalar.copy(st2[:], h2ps[:])
                    nc.sync.dma_start(
                        bass.AP(t2loc.tensor, it * 128 * RSTR,
                                [[RSTR, 128], [1, 42]]), st2[:])

            # ---------- P5: AllGather table2 ----------
            nc.gpsimd.collective_compute(
                "AllGather", mybir.AluOpType.bypass, replica_groups=groups,
                ins=[t2loc.opt()], outs=[t2full.opt()])

            # ---------- P6: edge pass layer 2 ----------
            for _ in range(rep):
                edge_pass(nc, tc, meta, 2, sidx_d, dloc_d, dlT_d,
                          t2full, t2loc, acc2, iota128, iota_p)

            # ---------- P7: normalize L2 -> log_softmax out ----------
            with tc.tile_pool(name="p7", bufs=3) as p7:
                for it in range(n_ptile):
                    ac = p7.tile([128, NW2], DT.float32, tag="ac2")
                    nc.sync.dma_start(
                        ac[:], bass.AP(acc2.tensor, it * 128 * ACC2W,
                                       [[ACC2W, 128], [1, NW2]]))
                    tr = p7.tile([128, 42], DT.bfloat16, tag="tr2")
                    nc.sync.dma_start(
                        tr[:], bass.AP(t2loc.tensor, it * 128 * RSTR,
                                       [[RSTR, 128], [1, 42]]))
                    es = p7.tile([128, 1], DT.float32, tag="es2")
                    nc.vector.tensor_tensor(out=es[:], in0=tr[:, 40:41],
                                            in1=tr[:, 41:42],
                                            op=mybir.AluOpType.add)
                    et = p7.tile([128, 1], DT.float32, tag="et2")
                    nc.vector.tensor_scalar_mul(et[:], es[:], NEG)
                    nc.vector.tensor_tensor(out=es[:], in0=es[:], in1=et[:],
                                            op=mybir.AluOpType.max)
                    ws = p7.tile([128, 1], DT.float32, tag="ws2")
                    nc.scalar.activation(ws[:], es[:],
                                         mybir.ActivationFunctionType.Exp)
                    wh = p7.tile([128, C2], DT.float32, tag="wh2")
                    nc.vector.tensor_scalar(
                        out=wh[:], in0=tr[:, 0:C2], scalar1=ws[:],
                        scalar2=None, op0=mybir.AluOpType.mult)
                    nc.vector.tensor_tensor(out=ac[:, 0:C2], in0=ac[:, 0:C2],
                                            in1=wh[:],
                                            op=mybir.AluOpType.add)
                    nc.vector.tensor_tensor(out=ac[:, C2:C2 + 1],
                                            in0=ac[:, C2:C2 + 1], in1=ws[:],
                                            op=mybir.AluOpType.add)
                    den = p7.tile([128, 1], DT.float32, tag="d2")
                    nc.vector.tensor_scalar_max(den[:], ac[:, C2:C2 + 1],
                                                1e-30)
                    rec = p7.tile([128, 1], DT.float32, tag="r2")
                    nc.vector.reciprocal(rec[:], den[:])
                    o2 = p7.tile([128, C2], DT.float32, tag="o2")
                    nc.vector.tensor_scalar(
                        out=o2[:], in0=ac[:, 0:C2], scalar1=rec[:],
                        scalar2=None, op0=mybir.AluOpType.mult)
                    nc.vector.tensor_tensor(out=o2[:], in0=o2[:], in1=b2sb[:],
                                            op=mybir.AluOpType.add)
                    mx = p7.tile([128, 1], DT.float32, tag="mx")
                    nc.vector.tensor_reduce(mx[:], o2[:],
                                            axis=mybir.AxisListType.X,
                                            op=mybir.AluOpType.max)
                    z = p7.tile([128, C2], DT.float32, tag="z")
                    nc.vector.tensor_scalar(
                        out=z[:], in0=o2[:], scalar1=mx[:], scalar2=None,
                        op0=mybir.AluOpType.subtract)
                    ez = p7.tile([128, C2], DT.float32, tag="ez")
                    se = p7.tile([128, 1], DT.float32, tag="se")
                    nc.scalar.activation(ez[:], z[:],
                                         mybir.ActivationFunctionType.Exp,
                                         accum_out=se[:])
                    lse = p7.tile([128, 1], DT.float32, tag="lse")
                    nc.scalar.activation(lse[:], se[:],
                                         mybir.ActivationFunctionType.Ln)
                    zo = p7.tile([128, C2], DT.float32, tag="zo")
                    nc.vector.tensor_scalar(
                        out=zo[:], in0=z[:], scalar1=lse[:], scalar2=None,
                        op0=mybir.AluOpType.subtract)
                    nc.sync.dma_start(out_d[128 * it:128 * (it + 1), :],
                                      zo[:])

    nc.compile()
    return nc


def edge_pass(nc, tc, meta, layer, sidx_d, dloc_d, dlT_d, tfull, tloc,
              acc, iota128, iota_p):
    ablate = set(os.environ.get("GAT_ABLATE", "").split(","))
    NST = meta["NST"]
    runs = meta["runs"]
    tblk = meta["tblk"]
    row = ROW1 if layer == 1 else ROW2          # src gather width
    adw = H1 if layer == 1 else 1               # a_dst values per node
    adcol = 72 if layer == 1 else 41            # a_dst column in the table
    nh = H1 if layer == 1 else 1
    nw = NW1 if layer == 1 else NW2
    accw = ACC1W if layer == 1 else ACC2W

    ntile_tot = NST * K
    # per-supertile distinct blocks (in order) and contiguous-bl runs
    st_blocks, st_bruns = [], []
    for s in range(NST):
        seg = tblk[s * K:(s + 1) * K]
        sb = []
        for b in seg:
            if not sb or sb[-1] != b:
                sb.append(b)
        st_blocks.append({b: i for i, b in enumerate(sb)})
        bruns = []
        for i, b in enumerate(sb):
            bl = b % NBPQ
            if bruns and bruns[-1][1] + bruns[-1][2] == bl and \
               sb[i - 1] == b - 1:
                bruns[-1][2] += 1
            else:
                bruns.append([i, bl, 1])   # [slot0, bl0, nrun]
        st_bruns.append(bruns)
    NBMAX = max(len(d) for d in st_blocks)
    # start/stop flags per tile
    start_f = [i == 0 or tblk[i] != tblk[i - 1] for i in range(ntile_tot)]
    stop_f = [i == ntile_tot - 1 or tblk[i] != tblk[i + 1]
              for i in range(ntile_tot)]
    # block -> flush group (groups of <=8 consecutive blocks, same quarter)
    def grp(b):
        q, bl = divmod(b, NBPQ)
        return q * ((NBPQ + 7) // 8) + bl // 8
    def grp_span(g):
        ng = (NBPQ + 7) // 8
        q, gl = divmod(g, ng)
        lo = gl * 8
        return q, lo, min(lo + 8, NBPQ)
    last_blk_of_grp = {}
    for i in range(ntile_tot):
        last_blk_of_grp[grp(tblk[i])] = tblk[i]

    from contextlib import ExitStack
    with ExitStack() as ctx:
        pm = ctx.enter_context(tc.tile_pool(name=f"e{layer}m", bufs=2))
        pg = ctx.enter_context(tc.tile_pool(name=f"e{layer}g", bufs=3))
        pw = ctx.enter_context(tc.tile_pool(name=f"e{layer}w", bufs=2))
        pb = ctx.enter_context(tc.tile_pool(name=f"e{layer}b", bufs=2))
        ps_blk = ctx.enter_context(
            tc.tile_pool(name=f"e{layer}ps", bufs=4, space="PSUM"))
        ps_ad = ctx.enter_context(
            tc.tile_pool(name=f"e{layer}pa", bufs=2, space="PSUM"))
        state = {}
        fix = {}
        if ablate & {"nogather", "novec"}:
            pfix = ctx.enter_context(
                tc.tile_pool(name=f"e{layer}fx", bufs=1))
            if "nogather" in ablate:
                fix["hs"] = pfix.tile([128, K * row], DT.bfloat16, name=f"fxhs{layer}")
                nc.vector.memset(fix["hs"][:], 0.5)
            if "novec" in ablate:
                fix["hw"] = pfix.tile([128, K * nw], DT.bfloat16, name=f"fxhw{layer}")
                nc.vector.memset(fix["hw"][:], 0.25)
                fix["spos"] = pfix.tile([128, K * 128], DT.bfloat16, name=f"fxsp{layer}")
                nc.vector.memset(fix["spos"][:], 0.0)

        def mm_stage(s, spos, hw):
            # per-tile matmul into the block PSUM; flush groups via accum-DMA
            for c in range(K):
                t = s * K + c
                b = tblk[t]
                if start_f[t]:
                    blkps = ps_blk.tile([128, nw], DT.float32, tag="blkps")
                    state["ps"] = blkps
                nc.tensor.matmul(
                    state["ps"][:],
                    lhsT=spos[:, 128 * c:128 * (c + 1)],
                    rhs=hw[:, nw * c:nw * (c + 1)],
                    start=start_f[t], stop=stop_f[t], skip_group_check=True)
                if stop_f[t]:
                    g = grp(b)
                    q, lo, hi = grp_span(g)
                    gt = state.get("g")
                    if gt is None:
                        gt = pb.tile([128, 8 * nw], DT.float32, tag="gacc",
                                     name=f"gacc_{layer}_{g}")
                        state["g"] = gt
                    bl = b % NBPQ
                    if "noflush" not in ablate:
                        nc.scalar.copy(
                            gt[:, nw * (bl - lo):nw * (bl - lo + 1)],
                            state["ps"][:])
                    if b == last_blk_of_grp[g]:
                        gs = hi - lo
                        gap = bass.AP(
                            gt[:].tensor, gt[:].offset,
                            [[gt[:].ap[0][0], BN], [nw, gs], [1, nw]])
                        if "noflush" not in ablate:
                            nc.gpsimd.dma_start(
                                bass.AP(acc.tensor, lo * BN * accw,
                                        [[accw, BN], [BN * accw, gs], [1, nw]]),
                                gap, accum_op=mybir.AluOpType.add)
                        state["g"] = None

        for s in range(NST):
            sidx = pm.tile([128, K * 8], DT.int16, tag="sidx")
            nc.sync.dma_start(sidx[:], sidx_d[s])
            dl = pm.tile([128, K], DT.bfloat16, tag="dl")
            nc.sync.dma_start(dl[:], dloc_d[s])

            # src gather (per quarter run), striped over the 4 SWDGE queues:
            # desc-gen/ring throughput is ~10.5 ns/desc serial per queue
            if "nogather" in ablate:
                hs = fix["hs"]
            else:
                hs = pg.tile([128, K * row], DT.bfloat16, tag="hs")
                for (c0, c1, q) in runs[s]:
                    n128 = c1 - c0
                    bnd = [c0 + (n128 * i) // 4 for i in range(5)]
                    for qi in range(4):
                        a, b = bnd[qi], bnd[qi + 1]
                        if b > a:
                            dma_gather_raw(
                                nc,
                                _brd(hs[:], [[row, b - a], [1, row]],
                                     off=row * a),
                                bass.AP(tfull.tensor, q * QS * RSTR,
                                        [[RSTR, QS], [1, row]]),
                                sidx[:, 8 * a:8 * b], (b - a) * 128, row,
                                RSTR, queue_num=qi)

            # e = asrc + adst ; leakyrelu ; w = exp(e)
            if "novec" in ablate:
                if "nomm" not in ablate:
                    mm_stage(s, fix["spos"], fix["hw"])
                continue

            # a_dst per edge, gather-free: replicate block-local dst ids to
            # all partitions (dense HWDGE DMA), build the TRANSPOSED one-hot
            # oneT[pos, (c,slot)] = (dl[slot,c] == pos) on DVE, then one small
            # PE matmul per tile broadcasts the block's dense a_dst rows
            # (udp) to edge slots in PSUM.
            dlr = pw.tile([128, K * 128], DT.bfloat16, tag="dlr")
            nc.sync.dma_start(
                dlr[:], bass.AP(dlT_d.tensor, s * K * 128,
                                [[0, 128], [1, K * 128]]))
            nc.vector.tensor_tensor(
                out=dlr[:], in0=dlr[:],
                in1=_brd(iota_p[:], [[0, K * 128]]),
                op=mybir.AluOpType.is_equal)
            udp = pw.tile([128, NBMAX * adw], DT.bfloat16, tag="udp")
            for (slot0, bl0, nrun) in st_bruns[s]:
                nc.sync.dma_start(
                    _brd(udp[0:BN, :], [[adw, nrun], [1, adw]],
                         off=slot0 * adw),
                    bass.AP(tloc.tensor, bl0 * BN * RSTR + adcol,
                            [[RSTR, BN], [BN * RSTR, nrun], [1, adw]]))
            half = K // 2
            psad = []
            for hf in range(2):
                pst = ps_ad.tile([128, half * adw], DT.float32, tag="psad")
                psad.append(pst)
                for c in range(hf * half, (hf + 1) * half):
                    b = tblk[s * K + c]
                    bs = st_blocks[s][b]
                    j = c - hf * half
                    nc.tensor.matmul(
                        pst[:, j * adw:(j + 1) * adw],
                        lhsT=dlr[0:BN, 128 * c:128 * (c + 1)],
                        rhs=udp[0:BN, bs * adw:(bs + 1) * adw],
                        start=True, stop=True, skip_group_check=True)

            e = pw.tile([128, K * nh], DT.float32, tag="e")
            for hf in range(2):
                nc.vector.tensor_tensor(
                    out=e[:, hf * half * nh:(hf + 1) * half * nh],
                    in0=_brd(hs[:], [[row, half], [1, nh]],
                             off=(64 if layer == 1 else 40) + hf * half * row),
                    in1=psad[hf][:],
                    op=mybir.AluOpType.add)
            tmp = pw.tile([128, K * nh], DT.float32, tag="etmp")
            nc.vector.tensor_scalar_mul(tmp[:], e[:], NEG)
            nc.vector.tensor_tensor(out=e[:], in0=e[:], in1=tmp[:],
                                    op=mybir.AluOpType.max)
            w = pw.tile([128, K * nh], DT.bfloat16, tag="w")
            nc.scalar.activation(w[:], e[:], mybir.ActivationFunctionType.Exp)

            # rhs = [h*w | w]
            hw = pw.tile([128, K * nw], DT.bfloat16, tag="hw")
            if layer == 1:
                nc.vector.tensor_tensor(
                    out=_brd(hw[:], [[nw, K], [8, 8], [1, 8]]),
                    in0=_brd(hs[:], [[row, K], [8, 8], [1, 8]]),
                    in1=_brd(w[:], [[nh, K], [1, 8], [0, 8]]),
                    op=mybir.AluOpType.mult)
                nc.vector.tensor_copy(
                    _brd(hw[:], [[nw, K], [1, 8]], off=64), w[:])
            else:
                nc.vector.tensor_tensor(
                    out=_brd(hw[:], [[nw, K], [1, C2]]),
                    in0=_brd(hs[:], [[row, K], [1, C2]]),
                    in1=_brd(w[:], [[1, K], [0, C2]]),
                    op=mybir.AluOpType.mult)
                nc.vector.tensor_copy(
                    _brd(hw[:], [[nw, K], [1, 1]], off=C2), w[:])

            # one-hot spos[slot, j*128+q] = (dl[slot,j] == q)
            spos = pw.tile([128, K * 128], DT.bfloat16, tag="spos")
            nc.vector.tensor_tensor(
                out=_brd(spos[:], [[128, K], [1, 128]]),
                in0=_brd(iota128[:], [[0, K], [1, 128]]),
                in1=_brd(dl[:], [[1, K], [0, 128]]),
                op=mybir.AluOpType.is_equal)

            if "nomm" not in ablate:
                mm_stage(s, spos, hw)


# =================== SPMD runner (bass2jax-based, with timing) ===================

def _run_spmd(nc, in_maps, n_timing_iters=0):
    """Execute the program on NCORES neuron devices via PJRT (axon)."""
    import jax
    from jax.sharding import Mesh, PartitionSpec
    from jax.experimental.shard_map import shard_map
    from concourse import bass2jax
    from concourse.bass2jax import _bass_exec_p, partition_id_tensor
    import time

    bass2jax.install_neuronx_cc_hook()
    assert nc.dbg_addr is None or not nc.dbg_callbacks

    in_names, out_names, out_avals, zero_outs = [], [], [], []
    partition_name = (nc.partition_id_tensor.name
                      if nc.partition_id_tensor else None)
    for alloc in nc.m.functions[0].allocations:
        if not isinstance(alloc, mybir.MemoryLocationSet):
            continue
        name = alloc.memorylocations[0].name
        if alloc.kind == "ExternalInput":
            if name != partition_name:
                in_names.append(name)
        elif alloc.kind == "ExternalOutput":
            out_names.append(name)
            shape = tuple(alloc.tensor_shape)
            dtype = mybir.dt.np(alloc.dtype)
            out_avals.append(jax.core.ShapedArray(shape, dtype))
            zero_outs.append(np.zeros(shape, dtype))
    n_params = len(in_names)
    all_in_names = in_names + out_names + (
        [partition_name] if partition_name else [])

    def _body(*args):
        operands = list(args)
        if partition_name is not None:
            operands.append(partition_id_tensor())
        return tuple(_bass_exec_p.bind(
            *operands,
            out_avals=tuple(out_avals),
            in_names=tuple(all_in_names),
            out_names=tuple(out_names),
            lowering_input_output_aliases=(),
            sim_require_finite=True,
            sim_require_nnan=True,
            nc=nc,
        ))

    devices = jax.devices()[:NCORES]
    mesh = Mesh(np.asarray(devices), ("core",))
    nin = n_params + len(out_names)
    fn = jax.jit(shard_map(_body, mesh=mesh,
                           in_specs=(PartitionSpec("core"),) * nin,
                           out_specs=(PartitionSpec("core"),) * len(out_names),
                           check_rep=False),
                 keep_unused=True)
    sh = jax.sharding.NamedSharding(mesh, PartitionSpec("core"))
    concat_in = [
        jax.device_put(np.concatenate(
            [np.asarray(in_maps[c][name]) for c in range(NCORES)], axis=0), sh)
        for name in in_names
    ]
    concat_zeros = [
        jax.device_put(np.zeros((NCORES * z.shape[0], *z.shape[1:]), z.dtype),
                       sh) for z in zero_outs
    ]
    out_arrs = jax.block_until_ready(fn(*concat_in, *concat_zeros))
    times = []
    for _ in range(n_timing_iters):
        t0 = time.perf_counter()
        r = jax.block_until_ready(fn(*concat_in, *concat_zeros))
        times.append(time.perf_counter() - t0)
        del r
    results = [
        {name: np.asarray(out_arrs[i]).reshape(NCORES, *out_avals[i].shape)[c]
         for i, name in enumerate(out_names)}
        for c in range(NCORES)
    ]
    return results, times


# =================== top-level entry ===================

def kernel(**inputs):
    edge_index = np.asarray(inputs["edge_index"])
    meta, per_core = preprocess(edge_index)
    wts = build_weight_inputs(
        np.asarray(inputs["W1"]), np.asarray(inputs["att_src1"]),
        np.asarray(inputs["att_dst1"]), np.asarray(inputs["bias1"]),
        np.asarray(inputs["W2"]), np.asarray(inputs["att_src2"]),
        np.asarray(inputs["att_dst2"]), np.asarray(inputs["bias2"]))
    x = np.asarray(inputs["x"], _f32)
    in_maps = []
    for c in range(NCORES):
        pc = per_core[c]
        xs = np.zeros((NPCP, F), _f32)
        xs[:NPC] = x[c * NPC:(c + 1) * NPC]
        in_maps.append(dict(
            x_sl=xs, W1=wts["W1"], A1=wts["A1"], W2=wts["W2"],
            att2=wts["att2"], b1r=wts["b1r"], b2r=wts["b2r"],
            sidx=pc["sidx"], dloc=pc["dloc"], dlT=pc["dlT"]))
    nc = build_program(meta)
    n_iters = int(os.environ.get("GAT_BENCH_ITERS", "0"))
    results, times = _run_spmd(nc, in_maps, n_timing_iters=n_iters)
    global LAST_TIMES
    LAST_TIMES = times
    out = np.zeros((N, C2), _f32)
    for c in range(NCORES):
        out[c * NPC:(c + 1) * NPC] = results[c]["out"][:NPC]
    return out


LAST_TIMES = []

